# revision 1
# baseline (speedup 1.0000x reference)
"""Trainium2 Bass kernel for nn_CausalAttention_76304388981436.

Full-input contract: kernel(**inputs) -> [2, 2048, 512] f32.

Sharding (8 cores, single SPMD program): core c = (batch b=c//4, head-pair
hp=c%4).  Each core computes attention for its 2 heads over the full 2048
sequence of its batch, producing a partial output  attnT_2h @ Wo[2h-rows]
[2048, 512]; host sums the 4 head-pair partials per batch and adds bo.

Device-side math per core (all matmuls fp32r, transposed-attention layout):
  qT[128i, 2048n] = wq2^T x^T   (wq2 pre-scaled by 1/8 on host)
  kT[128i, 2048n] = wk2^T x^T
  v [2048n, 2x65] = x wv2       (col 64 of each head block memset to 1.0 ->
                                 PV matmul also produces softmax denominator)
  P'[2048q, 1032] = qT_h^T rel_embT_rev   (rel table pre-reversed on host)
  PR dram [2048q, 2048c] fp16: P' chunks + clamp-value pads, laid out so the
      relative-position skew  bias[j,q] = P[q, clip(q-j,-512,512)+512]
      becomes a plain strided read:  flat = 2047*fq + pj + (1535 - A).
  S^T tile [128j, 512q] = kT_h^T qT_h (+ clamp-bias matmul | + skew-tile add)
  expS = exp(S^T)   (no max-subtraction: |logits| <~ 10, fp32-safe)
  outT_h [65, 512q] += v_ext_h^T expS   (row 64 = denominator)
  attnT_h = outT_h[0:64] * (1/denom broadcast)
  partial[2048q, 512] = attnT^T wo2
"""
import numpy as np
import ml_dtypes

import concourse.bass as bass
import concourse.mybir as mybir
import concourse.tile as tile
from concourse.bass_utils import run_bass_kernel_spmd

F32 = mybir.dt.float32
F32R = mybir.dt.float32r
F16 = mybir.dt.float16
BF16 = mybir.dt.bfloat16
AF = mybir.ActivationFunctionType

N = 2048          # sequence length
D = 512           # model dim
HD = 64           # head dim
NQC = 4           # q-chunks of 512
NJT = 16          # j-tiles of 128
W = 2048          # padded PR row width


def _split_multiwaits(nc):
    """This walrus build rejects >1 sync wait per instruction; split extra
    waits onto single-wait NoOps on the same engine just before it."""
    for func in nc.m.functions:
        for block in func.blocks:
            new_instrs = []
            for inst in block.instructions:
                si = inst.sync_info
                if si is not None and si.on_wait and len(si.on_wait) > 1:
                    waits = list(si.on_wait)
                    for w in waits[:-1]:
                        new_instrs.append(mybir.InstNoOp(
                            name=nc.get_next_instruction_name(),
                            engine=inst.engine,
                            bass_nofuse=True,
                            sync_info=mybir.SyncInfo(on_wait=[w], on_update=[]),
                        ))
                    si.on_wait = waits[-1:]
                new_instrs.append(inst)
            block.instructions = new_instrs


def _r(ap):
    return ap.bitcast(F32R)


def build_kernel():
    nc = bass.Bass("TRN2")
    xT = nc.dram_tensor("xT", [D, N], F32, kind="ExternalInput")
    wq2 = nc.dram_tensor("wq2", [D, 128], F32, kind="ExternalInput")
    wk2 = nc.dram_tensor("wk2", [D, 128], F32, kind="ExternalInput")
    wv2 = nc.dram_tensor("wv2", [D, 128], F32, kind="ExternalInput")
    wo2 = nc.dram_tensor("wo2", [128, D], F32, kind="ExternalInput")
    relT = nc.dram_tensor("relT", [128, W], BF16, kind="ExternalInput")
    relbc = nc.dram_tensor("relbc", [128, 256], BF16, kind="ExternalInput")
    ones2 = nc.dram_tensor("ones2", [128, 2, 1], F32, kind="ExternalInput")
    out = nc.dram_tensor("out", [N, D], F32, kind="ExternalOutput")

    with tile.TileContext(nc) as tc:
        _build_body(nc, tc, xT, wq2, wk2, wv2, wo2, relT, relbc, ones2, out)
    _split_multiwaits(nc)
    return nc


def _build_body(nc, tc, xT, wq2, wk2, wv2, wo2, relT, relbc, ones2, out):
    from contextlib import ExitStack
    ctx = ExitStack()
    consts = ctx.enter_context(tc.tile_pool(name="consts", bufs=1))
    qkv = ctx.enter_context(tc.tile_pool(name="qkv", bufs=1))
    pcast = ctx.enter_context(tc.tile_pool(name="pcast", bufs=6))
    skew = ctx.enter_context(tc.tile_pool(name="skew", bufs=6))
    exps = ctx.enter_context(tc.tile_pool(name="exps", bufs=18))
    small = ctx.enter_context(tc.tile_pool(name="small", bufs=4))
    outc = ctx.enter_context(tc.tile_pool(name="outc", bufs=2))
    psa = ctx.enter_context(tc.tile_pool(name="psa", bufs=4, space="PSUM"))
    pso = ctx.enter_context(tc.tile_pool(name="pso", bufs=2, space="PSUM"))
    pdram = ctx.enter_context(tc.tile_pool(name="pdram", bufs=1, space="DRAM"))

    # ---- load constants / inputs ----
    sxT = [consts.tile([128, N], F32, name=f"xT{i}", tag=f"xT{i}") for i in range(4)]
    for i in range(4):
        nc.sync.dma_start(out=_r(sxT[i][:]), in_=_r(xT[i * 128:(i + 1) * 128, :]))
    swq = [consts.tile([128, 128], F32, name=f"wq{i}", tag=f"wq{i}") for i in range(4)]
    swk = [consts.tile([128, 128], F32, name=f"wk{i}", tag=f"wk{i}") for i in range(4)]
    swv = [consts.tile([128, 128], F32, name=f"wv{i}", tag=f"wv{i}") for i in range(4)]
    for i in range(4):
        nc.sync.dma_start(out=_r(swq[i][:]), in_=_r(wq2[i * 128:(i + 1) * 128, :]))
        nc.sync.dma_start(out=_r(swk[i][:]), in_=_r(wk2[i * 128:(i + 1) * 128, :]))
        nc.sync.dma_start(out=_r(swv[i][:]), in_=_r(wv2[i * 128:(i + 1) * 128, :]))
    swo = consts.tile([128, D], F32, name="wo", tag="wo")
    nc.sync.dma_start(out=_r(swo[:]), in_=_r(wo2[:, :]))
    srelT = consts.tile([128, W], BF16, name="relT", tag="relT")
    nc.sync.dma_start(out=srelT[:], in_=relT[:, :])
    srelbc = consts.tile([128, 256], BF16, name="relbc", tag="relbc")
    nc.sync.dma_start(out=srelbc[:], in_=relbc[:, :])
    sones = consts.tile([128, 2, 1], F32, name="ones2", tag="ones2")
    nc.sync.dma_start(out=_r(sones[:]), in_=_r(ones2[:, :, :]))

    # ---- projections ----
    qT = qkv.tile([128, N], BF16, name="qT", tag="qT")
    kT = qkv.tile([128, N], BF16, name="kT", tag="kT")
    for nchunk in range(NQC):
        ns = slice(nchunk * 512, nchunk * 512 + 512)
        for dst, w in ((qT, swq), (kT, swk)):
            ps = psa.tile([128, 512], F32, name="ps", tag="ps")
            for c in range(4):
                nc.tensor.matmul(ps[:], _r(w[c][:]), _r(sxT[c][:, ns]),
                                 start=(c == 0), stop=(c == 3))
            nc.vector.tensor_copy(out=dst[:, ns], in_=ps[:])
    # v in natural layout with ones column per head block
    vt = [qkv.tile([128, 2, 65], BF16, name=f"v{t}", tag=f"v{t}") for t in range(NJT)]
    for t in range(NJT):
        nst = slice(t * 128, t * 128 + 128)
        ps = psa.tile([128, 128], F32, name="ps", tag="ps")
        for c in range(4):
            nc.tensor.matmul(ps[:], _r(sxT[c][:, nst]), _r(swv[c][:]),
                             start=(c == 0), stop=(c == 3))
        nc.vector.tensor_copy(out=vt[t][:, :, 64:65], in_=sones[:])
        nc.vector.tensor_copy(out=vt[t][:, :, 0:64],
                              in_=ps[:].rearrange("p (h d) -> p h d", h=2))

    # ---- P' phase: PR[q, col] = q_h . rel_ext[col]  -> PR dram (fp16) ----
    # rel_ext (host) already encodes reversal + clamp padding per column.
    prd = [pdram.tile([N, W], F16, name=f"pr{h}", tag=f"pr{h}") for h in range(2)]
    def emit_P():
        # both heads' K=64 matmuls adjacent: disjoint PE row-groups (base
        # partition 0 / 64) execute concurrently on the tiled array
        for qt in range(NJT):
            qs = slice(qt * 128, qt * 128 + 128)
            rows = slice(qt * 128, qt * 128 + 128)
            for ci in range(4):
                cs = slice(ci * 512, ci * 512 + 512)
                pss = []
                for h in range(2):
                    hs = slice(h * 64, h * 64 + 64)
                    ps = psa.tile([128, 512], F32, name="ps", tag="ps")
                    nc.tensor.matmul(ps[:], qT[hs, qs], srelT[hs, cs],
                                     start=True, stop=True,
                                     tile_position=(h * 64, 0))
                    pss.append(ps)
                for h in range(2):
                    ct = pcast.tile([128, 512], F16, name="pc", tag="pc")
                    nc.vector.tensor_copy(out=ct[:], in_=pss[h][:])
                    nc.gpsimd.dma_start(out=prd[h][rows, cs], in_=ct[:])

    # ---- attention ----
    attnT = qkv.tile([128, N], F32, name="attnT", tag="attnT")
    rdd = [pdram.tile([1, 512], F32, name=f"rdd{i}", tag=f"rdd{i}") for i in range(8)]
    def emit_attn():
        for qc in range(NQC):
            qs = slice(qc * 512, qc * 512 + 512)
            pos = [pso.tile([65, 512], F32, name="po", tag=f"po{h}")
                   for h in range(2)]
            ets = {0: [], 1: []}
            for jt in range(NJT):
                js = slice(jt * 128, jt * 128 + 128)
                A = qc * 512 + 512 - 128 * jt
                pss = []
                for h in range(2):
                    hs = slice(h * 64, h * 64 + 64)
                    ps = psa.tile([128, 512], F32, name="ps", tag="ps")
                    if A <= -512 or A >= 1152:
                        bc = 0 if A <= -512 else 128
                        nc.tensor.matmul(ps[:], kT[hs, js], qT[hs, qs],
                                         start=True, stop=False,
                                         tile_position=(h * 64, 0))
                        nc.tensor.matmul(ps[:], srelbc[hs, bc:bc + 128],
                                         qT[hs, qs], start=False, stop=True,
                                         tile_position=(h * 64, 0))
                    else:
                        nc.tensor.matmul(ps[:], kT[hs, js], qT[hs, qs],
                                         start=True, stop=True,
                                         tile_position=(h * 64, 0))
                    pss.append(ps)
                for h in range(2):
                    A2 = A
                    if not (A2 <= -512 or A2 >= 1152):
                        bt = skew.tile([128, 512], F16, name="skew", tag="skew")
                        srcap = bass.AP(tensor=prd[h].tensor,
                                        offset=prd[h].offset + qc * 512 * W + (1535 - A2),
                                        ap=[[2047, 512], [1, 128]])
                        nc.scalar.dma_start(out=bt[:], in_=srcap, transpose=True)
                        nc.vector.tensor_add(out=pss[h][:], in0=pss[h][:], in1=bt[:])
                    et = exps.tile([128, 512], BF16, name="expS", tag="expS")
                    nc.scalar.activation(out=et[:], in_=pss[h][:], func=AF.Exp)
                    ets[h].append(et)
            for jt in range(NJT):
                for h in range(2):
                    nc.tensor.matmul(pos[h][:], vt[jt][:, h, :], ets[h][jt][:],
                                     start=(jt == 0), stop=(jt == NJT - 1))
            for h in range(2):
                hs = slice(h * 64, h * 64 + 64)
                po = pos[h]
                rd = small.tile([1, 512], F32, name="rd", tag="rd")
                nc.vector.reciprocal(out=rd[:], in_=po[64:65, :])
                slot = h * 4 + qc
                nc.sync.dma_start(out=rdd[slot][:], in_=rd[:])
                rdb = small.tile([64, 512], F32, name="rdb", tag="rdb")
                bcast = bass.AP(tensor=rdd[slot].tensor, offset=rdd[slot].offset,
                                ap=[[0, 64], [1, 512]])
                nc.sync.dma_start(out=rdb[:], in_=bcast)
                nc.vector.tensor_mul(out=_r(attnT[hs, qs]), in0=po[0:64, :],
                                     in1=rdb[:])

    emit_P()
    emit_attn()

    # ---- output projection (partial over this core's 2 heads) ----
    for qt in range(NJT):
        qs = slice(qt * 128, qt * 128 + 128)
        ps = psa.tile([128, 512], F32, name="ps", tag="ps")
        nc.tensor.matmul(ps[:], _r(attnT[:, qs]), _r(swo[:]),
                         start=True, stop=True)
        ot = outc.tile([128, 512], F32, name="oc", tag="oc")
        nc.vector.tensor_copy(out=ot[:], in_=ps[:])
        nc.gpsimd.dma_start(out=out[qs, :], in_=ot[:])
    ctx.close()


_NC_CACHE = [None]


def _get_nc():
    if _NC_CACHE[0] is None:
        _NC_CACHE[0] = build_kernel()
    return _NC_CACHE[0]


def make_in_maps(x, Wq, Wkv, Wo, bo, rel_emb):
    xT = [np.ascontiguousarray(x[b].T).astype(np.float32) for b in range(2)]
    cols = np.arange(W)
    idx = np.clip(1535 - cols, 0, 1024)
    relT = np.empty((128, W), np.float32)
    relT[0:64] = rel_emb[idx].T
    relT[64:128] = relT[0:64]
    relT = relT.astype(ml_dtypes.bfloat16)          # reversed rel table
    relbc = np.empty((128, 256), np.float32)
    relbc[0:64, 0:128] = rel_emb[0][:, None]       # clamp-low value
    relbc[0:64, 128:256] = rel_emb[1024][:, None]  # clamp-high value
    relbc[64:128] = relbc[0:64]
    relbc = relbc.astype(ml_dtypes.bfloat16)
    in_maps = []
    for c in range(8):
        b, hp = c // 4, c % 4
        cs = slice(hp * 128, hp * 128 + 128)
        in_maps.append({
            "xT": xT[b],
            "wq2": np.ascontiguousarray(Wq[:, cs] / 8.0).astype(np.float32),
            "wk2": np.ascontiguousarray(Wkv[:, :512][:, cs]).astype(np.float32),
            "wv2": np.ascontiguousarray(Wkv[:, 512:][:, cs]).astype(np.float32),
            "wo2": np.ascontiguousarray(Wo[cs, :]).astype(np.float32),
            "relT": relT,
            "relbc": relbc,
            "ones2": np.ones((128, 2, 1), np.float32),
        })
    return in_maps


def run(x, Wq, Wkv, Wo, bo, rel_emb, trace=False, trace_cores=None):
    nc = _get_nc()
    in_maps = make_in_maps(x, Wq, Wkv, Wo, bo, rel_emb)
    res = run_bass_kernel_spmd(nc, in_maps, core_ids=list(range(8)),
                               trace=trace, trace_cores=trace_cores)
    out = np.zeros((2, N, D), np.float32)
    for c in range(8):
        out[c // 4] += res.results[c]["out"]
    out += np.asarray(bo, np.float32)[None, None, :]
    return out, res


def kernel(x, Wq, Wkv, Wo, bo, rel_emb):
    out, _ = run(np.asarray(x), np.asarray(Wq), np.asarray(Wkv),
                 np.asarray(Wo), np.asarray(bo), np.asarray(rel_emb))
    return out



# revision 5
# speedup vs baseline: 1.4387x; 1.4387x over previous
"""Trainium2 Bass kernel for nn_CausalAttention_76304388981436.

Full-input contract: kernel(**inputs) -> [2, 2048, 512] f32.

Sharding (8 cores, single SPMD program): core c = (batch b=c//4, head-pair
hp=c%4).  Each core computes attention for its 2 heads over the full 2048
sequence of its batch, producing per-head UNNORMALIZED projected numerators
outh[h] = (sum_j exp(S-11) vT)^T @ Wo[h-rows]  [2048, 512] f16 plus the
softmax denominators dens [8, 512] f16; the host divides per head, sums the
4 head-pair partials per batch and adds bo.

Device-side math per core (transposed-attention layout, heads packed at
partitions 0-63 / 64-127 of the PE array):
  qT[128i, 2048n] = wq2^T x^T   (wq2 pre-scaled by 1/8 on host)
  kT[128i, 2048n] = wk2^T x^T
  v [2048n, 2x65] = x wv2       (col 64 of each head block memset to 1.0 ->
                                 PV matmul also produces softmax denominator)
  P' tiles [128q, 512s] = qT_h^T rel_embT_rev  (rel table pre-reversed on
      host; only the live s-chunks per q-chunk are computed/stored)
  PR dram per (h, qc) [512q, 2048s] fp16: bias[j,q] = PR[q, j-q+1023]
  skew read: ONE transposing DMA per (h, qc) with source AP
      [[2047, 512], [1, 128*njt]] (contiguous 2-3KB runs) -> SBUF
      [128j, njt, 512q] covering every in-band j-tile of the q-chunk.
  S^T tile [128j, 512q] = kT_h^T qT_h; bias added via a SECOND accumulating
      matmul ident_f16^T @ bias_tile (in-band) or the clamp-value matmul
      (out-of-band), all into the same PSUM accumulation group.
  expS = exp(S^T - 11)  one ACT op per [128, 1024] PSUM bank-pair
  outT_h [65, 512q] += v_ext_h^T expS   (row 64 = denominator)
  outh[h] [2048q, 512] = numT_h^T wo2_h  (unnormalized, fp16 out)
"""
import numpy as np
import ml_dtypes

import concourse.bass as bass
import concourse.mybir as mybir
import concourse.tile as tile
from concourse.bass_utils import run_bass_kernel_spmd

F32 = mybir.dt.float32
F32R = mybir.dt.float32r
F16 = mybir.dt.float16
BF16 = mybir.dt.bfloat16
AF = mybir.ActivationFunctionType

N = 2048          # sequence length
D = 512           # model dim
HD = 64           # head dim
NQC = 4           # q-chunks of 512
NJT = 16          # j-tiles of 128
W = 2048          # PR row width
SHIFT = -11.0     # exp(logit + SHIFT): keeps num/den in fp16 range

# per q-chunk: (jt_min, njt) of in-band j-tiles (-512 < A < 1152,
# A = 512*(qc+1) - 128*jt)
IN_BAND = {0: (0, 8), 1: (0, 12), 2: (4, 12), 3: (8, 8)}
# per q-chunk: live 512-wide s-chunks of PR (others never read)
LIVE_CI = {0: (1, 2, 3), 1: (0, 1, 2, 3), 2: (0, 1, 2, 3), 3: (0, 1, 2)}


def _split_multiwaits(nc):
    """This walrus build rejects >1 sync wait per instruction; split extra
    waits onto single-wait NoOps on the same engine just before it."""
    for func in nc.m.functions:
        for block in func.blocks:
            new_instrs = []
            for inst in block.instructions:
                si = inst.sync_info
                if si is not None and si.on_wait and len(si.on_wait) > 1:
                    waits = list(si.on_wait)
                    for w in waits[:-1]:
                        new_instrs.append(mybir.InstNoOp(
                            name=nc.get_next_instruction_name(),
                            engine=inst.engine,
                            bass_nofuse=True,
                            sync_info=mybir.SyncInfo(on_wait=[w], on_update=[]),
                        ))
                    si.on_wait = waits[-1:]
                new_instrs.append(inst)
            block.instructions = new_instrs


def _r(ap):
    return ap.bitcast(F32R)


def build_kernel():
    nc = bass.Bass("TRN2")
    xT = nc.dram_tensor("xT", [D, N], F32, kind="ExternalInput")
    wq2 = nc.dram_tensor("wq2", [D, 128], F32, kind="ExternalInput")
    wk2 = nc.dram_tensor("wk2", [D, 128], F32, kind="ExternalInput")
    wv2 = nc.dram_tensor("wv2", [D, 128], F32, kind="ExternalInput")
    wo2 = nc.dram_tensor("wo2", [128, D], F32, kind="ExternalInput")
    relT = nc.dram_tensor("relT", [128, W], BF16, kind="ExternalInput")
    relbc = nc.dram_tensor("relbc", [128, 256], BF16, kind="ExternalInput")
    ones2 = nc.dram_tensor("ones2", [128, 2, 1], F32, kind="ExternalInput")
    ident = nc.dram_tensor("ident", [128, 128], F16, kind="ExternalInput")
    outh = nc.dram_tensor("outh", [2, N, D], F16, kind="ExternalOutput")
    dens = nc.dram_tensor("dens", [8, 512], F16, kind="ExternalOutput")

    with tile.TileContext(nc) as tc:
        _build_body(nc, tc, xT, wq2, wk2, wv2, wo2, relT, relbc, ones2,
                    ident, outh, dens)
    _split_multiwaits(nc)
    return nc


def _build_body(nc, tc, xT, wq2, wk2, wv2, wo2, relT, relbc, ones2, ident,
                outh, dens):
    from contextlib import ExitStack
    ctx = ExitStack()
    consts = ctx.enter_context(tc.tile_pool(name="consts", bufs=1))
    qkv = ctx.enter_context(tc.tile_pool(name="qkv", bufs=1))
    pc = ctx.enter_context(tc.tile_pool(name="pc", bufs=3))
    skew = ctx.enter_context(tc.tile_pool(name="skew", bufs=2))
    exps = ctx.enter_context(tc.tile_pool(name="exps", bufs=12))
    outc = ctx.enter_context(tc.tile_pool(name="outc", bufs=3))
    dent = ctx.enter_context(tc.tile_pool(name="dent", bufs=4))
    psa = ctx.enter_context(tc.tile_pool(name="psa", bufs=1, space="PSUM"))
    pos = ctx.enter_context(tc.tile_pool(name="pos", bufs=1, space="PSUM"))
    pp = ctx.enter_context(tc.tile_pool(name="pp", bufs=1, space="PSUM"))
    pdram = ctx.enter_context(tc.tile_pool(name="pdram", bufs=1, space="DRAM"))

    # ---- load constants / inputs ----
    sxT = [consts.tile([128, N], F32, name=f"xT{i}", tag=f"xT{i}") for i in range(4)]
    for i in range(4):
        nc.sync.dma_start(out=_r(sxT[i][:]), in_=_r(xT[i * 128:(i + 1) * 128, :]))
    swq = [consts.tile([128, 128], F32, name=f"wq{i}", tag=f"wq{i}") for i in range(4)]
    swk = [consts.tile([128, 128], F32, name=f"wk{i}", tag=f"wk{i}") for i in range(4)]
    swv = [consts.tile([128, 128], F32, name=f"wv{i}", tag=f"wv{i}") for i in range(4)]
    for i in range(4):
        nc.sync.dma_start(out=_r(swq[i][:]), in_=_r(wq2[i * 128:(i + 1) * 128, :]))
        nc.sync.dma_start(out=_r(swk[i][:]), in_=_r(wk2[i * 128:(i + 1) * 128, :]))
        nc.sync.dma_start(out=_r(swv[i][:]), in_=_r(wv2[i * 128:(i + 1) * 128, :]))
    swo = consts.tile([128, D], F32, name="wo", tag="wo")
    nc.sync.dma_start(out=_r(swo[:]), in_=_r(wo2[:, :]))
    srelT = consts.tile([128, W], BF16, name="relT", tag="relT")
    nc.sync.dma_start(out=srelT[:], in_=relT[:, :])
    srelbc = consts.tile([128, 256], BF16, name="relbc", tag="relbc")
    nc.sync.dma_start(out=srelbc[:], in_=relbc[:, :])
    sones = consts.tile([128, 2, 1], F32, name="ones2", tag="ones2")
    nc.sync.dma_start(out=_r(sones[:]), in_=_r(ones2[:, :, :]))
    sident = consts.tile([128, 128], F16, name="ident", tag="ident")
    nc.sync.dma_start(out=sident[:], in_=ident[:, :])
    sbias = consts.tile([128, 1], F32, name="sbias", tag="sbias")
    nc.gpsimd.memset(sbias[:], SHIFT)

    # rotating psum slots for the projection phase (attention not started)
    def ps_slot(i):
        tag = ("pp", "psa0", "psa1")[i % 3]
        pool = pp if tag == "pp" else psa
        return pool.tile([128, 1024], F32, name=tag, tag=tag)

    # ---- projections ----
    qT = qkv.tile([128, N], BF16, name="qT", tag="qT")
    kT = qkv.tile([128, N], BF16, name="kT", tag="kT")
    slot = 0
    for nchunk in range(NQC):
        ns = slice(nchunk * 512, nchunk * 512 + 512)
        for dst, w in ((qT, swq), (kT, swk)):
            half = ps_slot(slot)[:, 0:512]
            slot += 1
            for c in range(4):
                nc.tensor.matmul(half, _r(w[c][:]), _r(sxT[c][:, ns]),
                                 start=(c == 0), stop=(c == 3))
            nc.vector.tensor_copy(out=dst[:, ns], in_=half)
    # v in natural layout with ones column per head block
    vt = [qkv.tile([128, 2, 65], BF16, name=f"v{t}", tag=f"v{t}") for t in range(NJT)]
    for t in range(NJT):
        nst = slice(t * 128, t * 128 + 128)
        half = ps_slot(slot)[:, 0:128]
        slot += 1
        for c in range(4):
            nc.tensor.matmul(half, _r(sxT[c][:, nst]), _r(swv[c][:]),
                             start=(c == 0), stop=(c == 3))
        nc.vector.tensor_copy(out=vt[t][:, :, 64:65], in_=sones[:])
        nc.vector.tensor_copy(out=vt[t][:, :, 0:64],
                              in_=half.rearrange("p (h d) -> p h d", h=2))

    # ---- per-(h, qc) PR dram tensors ----
    prd = {(h, qc): pdram.tile([512, W], F16, name=f"pr{h}_{qc}",
                               tag=f"pr{h}_{qc}")
           for h in range(2) for qc in range(NQC)}

    def p_units_for(qc):
        """Fine-grained closures producing P'(qc): one per (qt, ci) matmul+
        copy step, plus one per qt for the PR write DMAs."""
        if qc >= NQC:
            return []
        units = []
        cis = LIVE_CI[qc]
        state = {}
        for qt_local in range(4):
            qt = 4 * qc + qt_local

            def start_qt(qt=qt):
                state[qt] = pc.tile([128, 2, W], F16, name="pct", tag="pct")

            units.append(start_qt)
            for ci in cis:
                def do_ci(qt=qt, ci=ci):
                    qs = slice(qt * 128, qt * 128 + 128)
                    cs = slice(ci * 512, ci * 512 + 512)
                    ps = pp.tile([128, 1024], F32, name="pp", tag="pp")
                    for h in range(2):
                        hs = slice(h * 64, h * 64 + 64)
                        nc.tensor.matmul(ps[:, h * 512:h * 512 + 512],
                                         qT[hs, qs], srelT[hs, cs],
                                         start=True, stop=True,
                                         tile_position=(h * 64, 0))
                    nc.vector.tensor_copy(
                        out=state[qt][:, :, cs],
                        in_=ps[:].rearrange("p (h s) -> p h s", h=2))

                units.append(do_ci)

            def write_qt(qt=qt, qt_local=qt_local):
                lo, hi = cis[0] * 512, cis[-1] * 512 + 512
                rows = slice(qt_local * 128, qt_local * 128 + 128)
                pct = state[qt]
                for h in range(2):
                    nc.gpsimd.dma_start(out=prd[(h, qc)][rows, lo:hi],
                                        in_=pct[:, h, lo:hi])

            units.append(write_qt)
        return units

    def emit_skew_read(qc):
        """One transposing DMA per head covering all in-band j-tiles."""
        jt_min, njt = IN_BAND[qc]
        tiles = {}
        for h in range(2):
            skt = skew.tile([128, 12, 512], F16, name="skt", tag=f"skt{h}")
            t = prd[(h, qc)]
            src = bass.AP(tensor=t.tensor,
                          offset=t.offset + 128 * jt_min - 512 * qc + 1023,
                          ap=[[2047, 512], [1, 128 * njt]])
            nc.sync.dma_start(out=skt[:, 0:njt, :], in_=src, transpose=True)
            tiles[h] = skt
        return tiles

    # ---- attention ----
    # ah holds both heads' unnormalized numerators: rows 0-63 h0, 64-127 h1
    ah = qkv.tile([128, N], F32, name="ah", tag="ah")

    def emit_attn(qc, skt, p_units):
        jt_min, njt = IN_BAND[qc]
        qs = slice(qc * 512, qc * 512 + 512)
        pot = [pos.tile([65, 512], F32, name="po", tag=f"po{h}")
               for h in range(2)]
        ets = {0: [None] * 8, 1: [None] * 8}
        pu = iter(p_units)

        def emit_pv(p):
            for h in range(2):
                for idx in range(2):
                    jt = 2 * p + idx
                    nc.tensor.matmul(pot[h][:], vt[jt][:, h, :],
                                     ets[h][p][:, idx * 512:idx * 512 + 512],
                                     start=(jt == 0), stop=(jt == NJT - 1))

        for p in range(8):
            pst = {}
            for h in range(2):
                pst[h] = psa.tile([128, 1024], F32, name="psa", tag=f"psa{h}")
            # S matmuls: heads adjacent -> concurrent PE row groups
            for idx in range(2):
                jt = 2 * p + idx
                js = slice(jt * 128, jt * 128 + 128)
                for h in range(2):
                    hs = slice(h * 64, h * 64 + 64)
                    nc.tensor.matmul(pst[h][:, idx * 512:idx * 512 + 512],
                                     kT[hs, js], qT[hs, qs],
                                     start=True, stop=False,
                                     tile_position=(h * 64, 0))
            # bias adds
            for idx in range(2):
                jt = 2 * p + idx
                A = qc * 512 + 512 - 128 * jt
                in_band = not (A <= -512 or A >= 1152)
                for h in range(2):
                    hs = slice(h * 64, h * 64 + 64)
                    half = pst[h][:, idx * 512:idx * 512 + 512]
                    if in_band:
                        nc.tensor.matmul(half, sident[:],
                                         skt[h][:, jt - jt_min, :],
                                         start=False, stop=True)
                    else:
                        bc = 0 if A <= -512 else 128
                        nc.tensor.matmul(half, srelbc[hs, bc:bc + 128],
                                         qT[hs, qs], start=False, stop=True,
                                         tile_position=(h * 64, 0))
            for h in range(2):
                et = exps.tile([128, 1024], BF16, name="expS", tag="expS")
                nc.scalar.activation(out=et[:], in_=pst[h][:], func=AF.Exp,
                                     bias=sbias[:])
                ets[h][p] = et
            if p > 0:
                emit_pv(p - 1)
            # interleave P'(qc+1) production
            for _ in range(3):
                u = next(pu, None)
                if u is not None:
                    u()
        emit_pv(7)
        for u in pu:
            u()
        # numerators -> ah (f32), denominators -> dram (f16)
        for h in range(2):
            hs = slice(h * 64, h * 64 + 64)
            nc.vector.tensor_copy(out=_r(ah[hs, qs]), in_=_r(pot[h][0:64, :]))
            dt = dent.tile([1, 512], F16, name="den", tag="den")
            nc.vector.tensor_copy(out=dt[:], in_=pot[h][64:65, :])
            nc.gpsimd.dma_start(out=dens[h * 4 + qc:h * 4 + qc + 1, :],
                                in_=dt[:])

    # ---- pipeline: P'(0) up front, then attn(qc) || P'(qc+1) ----
    for u in p_units_for(0):
        u()
    for qc in range(NQC):
        skt = emit_skew_read(qc)
        emit_attn(qc, skt, p_units_for(qc + 1))

    # ---- output projection (per head, unnormalized) ----
    for qt in range(NJT):
        qs = slice(qt * 128, qt * 128 + 128)
        ps = psa.tile([128, 1024], F32, name="psa", tag=f"psa{qt % 2}")
        for h in range(2):
            hs = slice(h * 64, h * 64 + 64)
            nc.tensor.matmul(ps[:, h * 512:h * 512 + 512],
                             _r(ah[hs, qs]), _r(swo[hs, :]),
                             start=True, stop=True,
                             tile_position=(h * 64, 0))
        ot = outc.tile([128, 1024], F16, name="oc", tag="oc")
        nc.vector.tensor_copy(out=ot[:], in_=ps[:])
        for h in range(2):
            nc.gpsimd.dma_start(out=outh[h, qs, :],
                                in_=ot[:, h * 512:h * 512 + 512])
    ctx.close()


_NC_CACHE = [None]


def _get_nc():
    if _NC_CACHE[0] is None:
        _NC_CACHE[0] = build_kernel()
    return _NC_CACHE[0]


def make_in_maps(x, Wq, Wkv, Wo, bo, rel_emb):
    xT = [np.ascontiguousarray(x[b].T).astype(np.float32) for b in range(2)]
    cols = np.arange(W)
    idx = np.clip(1535 - cols, 0, 1024)
    relT = np.empty((128, W), np.float32)
    relT[0:64] = rel_emb[idx].T
    relT[64:128] = relT[0:64]
    relT = relT.astype(ml_dtypes.bfloat16)          # reversed rel table
    relbc = np.empty((128, 256), np.float32)
    relbc[0:64, 0:128] = rel_emb[0][:, None]       # clamp-low value
    relbc[0:64, 128:256] = rel_emb[1024][:, None]  # clamp-high value
    relbc[64:128] = relbc[0:64]
    relbc = relbc.astype(ml_dtypes.bfloat16)
    ident = np.eye(128, dtype=np.float16)
    in_maps = []
    for c in range(8):
        b, hp = c // 4, c % 4
        cs = slice(hp * 128, hp * 128 + 128)
        in_maps.append({
            "xT": xT[b],
            "wq2": np.ascontiguousarray(Wq[:, cs] / 8.0).astype(np.float32),
            "wk2": np.ascontiguousarray(Wkv[:, :512][:, cs]).astype(np.float32),
            "wv2": np.ascontiguousarray(Wkv[:, 512:][:, cs]).astype(np.float32),
            "wo2": np.ascontiguousarray(Wo[cs, :]).astype(np.float32),
            "relT": relT,
            "relbc": relbc,
            "ones2": np.ones((128, 2, 1), np.float32),
            "ident": ident,
        })
    return in_maps


def run(x, Wq, Wkv, Wo, bo, rel_emb, trace=False, trace_cores=None):
    nc = _get_nc()
    in_maps = make_in_maps(x, Wq, Wkv, Wo, bo, rel_emb)
    res = run_bass_kernel_spmd(nc, in_maps, core_ids=list(range(8)),
                               trace=trace, trace_cores=trace_cores)
    out = np.zeros((2, N, D), np.float32)
    for c in range(8):
        b = c // 4
        num = np.asarray(res.results[c]["outh"], np.float32)   # [2, N, D]
        den = np.asarray(res.results[c]["dens"], np.float32)   # [8, 512]
        for h in range(2):
            out[b] += num[h] / den[h * 4:(h + 1) * 4].reshape(N)[:, None]
    out += np.asarray(bo, np.float32)[None, None, :]
    return out, res


def kernel(x, Wq, Wkv, Wo, bo, rel_emb):
    out, _ = run(np.asarray(x), np.asarray(Wq), np.asarray(Wkv),
                 np.asarray(Wo), np.asarray(bo), np.asarray(rel_emb))
    return out
